# revision 73
# baseline (speedup 1.0000x reference)
"""3-layer GAT (GATConv x3 + linear head + softmax) on 8 Trainium2 NeuronCores.

Strategy: nodes partitioned into 8 contiguous blocks (2500 real + 60 pad rows
per core -> 2560 = 20 tiles of 128). Edges assigned to the core owning their
dst node, sorted by dst tile. Per layer:
  1. dense phase (per 128-row tile): h_aug = x @ W' where W' = [W | W@att_src |
     W@att_dst] (attention halves folded into the matmul on host, fp64). lhsT
     for layer 0 is host-pre-transposed; for layers 1-2 it is PE-transposed
     from the previous layer's relu output and kept resident in SBUF (no DRAM
     round trip, no DMA transposes). Layer l+1's dense tile t is EMITTED inline
     right after finalize(l, t) so it truly overlaps layer l's edge phase in
     every engine's (in-order) instruction stream. h_aug rows (bf16 h | raw
     fp32 a_s via bitcast) go to h_local DRAM; a_d to a resident bf16 table.
  2. one AllGather per layer into a Shared h_full tile, fired right after the
     last dense tile. (Chunked/half AGs were tried and are net losses: the
     collective executes on -- and blocks -- the in-order gpsimd queue that
     also issues the gathers.)
  3. zd pre-pass (overlaps the edge tail / AG wait): per group, DMA the
     host-shipped transposed one-hot indicator and matmul it with adloc to
     give every edge's a_d -> zdsb. No AG dependency; the zd PSUM slice is
     double-buffered by group parity so groups pipeline.
  4. edge phase per 1024-edge group (SWDGE dma_gather, 4 queues, 4-deep hr
     buffering -- the Q7 descriptor generation at ~7.6ns/edge is the edge
     bottleneck, so pipeline depth matters): one gather pulls src rows
     (2304B/edge); z = a_s + a_d, alpha = leaky(z), w = exp(alpha); the
     forward one-hot ifg is built on-device (iota is_equal dstrel); per-head
     weighted indicators (wind = w * ifg) built with head 0 on ScalarE
     (per-partition-scale activation) and heads 1-3 in one batched DVE op;
     per 128-edge subchunk 4x256-col matmuls scatter-add w*h, and per-tile
     runs accumulate [w | alpha] partials in a scratch PSUM bank that DVE
     folds into an SBUF accumulator.
  5. tile finalize: out = (num * exp(-m)/(exp(-m)*s + 1e-16)) + b, relu; then
     PE-transpose into the next layer's resident lhsT. The exp(-m) factor
     reproduces the reference-as-executed softmax shift exactly (segment_max
     lowers to segment_sum on this platform).
Final layer fuses the fc head + row softmax; outputs concatenated on host.
PSUM layout (8 banks): agg h 2x2 | dense ph 3x1 (512-col generations, also
transposes + fc) | scratch bank (zd x2 parity slices + [w|alpha] partials).
"""
import sys

sys.path.insert(0, "/opt/trn_rl_repo")

import ml_dtypes
import numpy as np

N = 20000
E = 320000
IN = 131
INP = 256          # IN padded to 2 k-chunks
H = 4
C = 256
HC = 1024
WA = 1032          # W' columns: 1024 h | 4 a_s | 4 a_d
OUT = 6
NEG = 0.2
NCORES = 8
RPC = 2500         # real rows per core
PR = 2560          # padded rows per core (20 tiles of 128)
TILES = PR // 128
HAUG = 1152        # bf16 h_aug row: 1024 h | 8 (4 fp32 a_s) | 8 spare | pad
CHUNKS = 1         # src halves == AllGather chunks. Collectives execute on
                   # (and block) the in-order gpsimd queue that also issues
                   # the gathers, so a mid-edge half-AG stalls the remaining
                   # edge phase; a single AG per layer is net faster.
SEGCAP = 8         # subchunks (x128 edges) per gather group


def _nchunks():
    return CHUNKS if TILES % CHUNKS == 0 else (2 if TILES % 2 == 0 else 1)


def _schedule(edge_index: np.ndarray):
    """Partition + sort edges; build per-core device arrays and the shared
    compile-time segment schedule: a tile-major subchunk stream cut into
    SEGCAP-subchunk gather groups that may span dst-tile boundaries (at most
    2 PSUM agg tiles are ever open)."""
    nch = _nchunks()
    hpr = PR // nch             # rows per src half
    WT = 1                      # dst tiles per window

    src_g = np.concatenate([edge_index[0], np.arange(N, dtype=np.int64)])
    dst_g = np.concatenate([edge_index[1], np.arange(N, dtype=np.int64)])
    dst_l = dst_g % RPC                   # local dst row in [0, RPC)
    core = dst_g // RPC

    # Per-core row permutation: bin-pack nodes into tiles balanced by
    # incoming-edge count, so the shared (max-across-cores) subchunk schedule
    # carries less padding. inv[c][orig_local] = permuted local row.
    inv = np.zeros((NCORES, RPC), np.int64)
    for c in range(NCORES):
        deg = np.bincount(dst_l[core == c], minlength=RPC)
        order = np.argsort(-deg, kind="stable")
        tsum = np.zeros(TILES, np.int64)
        tfill = np.zeros(TILES, np.int64)
        for j in order:
            open_t = np.flatnonzero(tfill < 128)
            tt = open_t[np.argmin(tsum[open_t])]
            inv[c, j] = tt * 128 + tfill[tt]
            tfill[tt] += 1
            tsum[tt] += deg[j]

    src_c = src_g // RPC
    src_l = inv[src_c, src_g % RPC]
    half = src_l // hpr
    src_d = src_c * hpr + (src_l % hpr)   # row id within the half tensor

    per_core = []
    counts = np.zeros((NCORES, TILES, nch), np.int64)
    for c in range(NCORES):
        sel = core == c
        s = src_d[sel]
        q = half[sel]
        dl = inv[c, dst_l[sel]]
        t = dl // 128
        order = np.lexsort((dl, q, t))
        s, q, dl, t = s[order], q[order], dl[order], t[order]
        np.add.at(counts[c], (t, q), 1)
        per_core.append((s, q, dl, t))

    k = np.ceil(counts.max(axis=0) / 128).astype(np.int64)   # [TILES, nch]

    # stream: per window of WT tiles, per half, the (tile, half) buckets
    segs = []       # (q, [(tile, k_t), ...], base_sub)
    tile_of_sub = []
    base = {}       # (t, q) -> slot base
    # tile-major subchunk stream (each tile >=1 subchunk), padded to a
    # multiple of SEGCAP; cut into SEGCAP-subchunk gather groups that may
    # span dst-tile boundaries (the per-tile PSUM runs handle that)
    assert nch == 1
    kt = np.maximum(1, k[:, 0])
    kt[TILES - 1] += (-int(kt.sum())) % SEGCAP
    for t in range(TILES):
        base[(t, 0)] = len(tile_of_sub) * 128
        tile_of_sub.extend([t] * int(kt[t]))
    for g0 in range(0, len(tile_of_sub), SEGCAP):
        chunk = tile_of_sub[g0:g0 + SEGCAP]
        tk = []
        for t in chunk:
            if tk and tk[-1][0] == t:
                tk[-1][1] += 1
            else:
                tk.append([t, 1])
        segs.append((0, [tuple(x) for x in tk], g0))
    total_sub = len(tile_of_sub)
    tile_of_sub = np.asarray(tile_of_sub)

    srcA = np.zeros((NCORES, total_sub * 128), np.int16)
    rel = np.full((NCORES, total_sub * 128), 200.0, np.float32)
    for c in range(NCORES):
        s, q, dl, t = per_core[c]
        for (tt, qq), b in base.items():
            m = (t == tt) & (q == qq)
            n = int(m.sum())
            srcA[c, b:b + n] = s[m].astype(np.int16)
            rel[c, b:b + n] = (dl[m] - tt * 128).astype(np.float32)

    # per-segment 16-partition wrap (8x replicated), concatenated columns:
    # segment at base_sub owns isrc cols [base_sub*8, (base_sub+nsb)*8)
    isrc = np.zeros((NCORES, 128, total_sub * 8), np.int16)
    for q, tk, base_sub in segs:
        nsb = sum(kk for _, kk in tk)
        n = nsb * 128
        b = base_sub * 128
        a = srcA[:, b:b + n]
        wv = a.reshape(NCORES, n // 16, 16).transpose(0, 2, 1)
        isrc[:, :, base_sub * 8:(base_sub + nsb) * 8] = np.tile(wv, (1, 8, 1))

    # dstrel plane [128, nsub]: [p, s] = rel dst of edge s*128+p (bf16-exact)
    drel = rel.reshape(NCORES, total_sub, 128).transpose(0, 2, 1)
    drel = drel.astype(ml_dtypes.bfloat16).copy()
    # transposed one-hot indicator for the zd pre-pass:
    # indT[j, s*128+e] = 1 iff edge (s,e)'s relative dst row == j
    indT = np.zeros((NCORES, 128, total_sub * 128), ml_dtypes.bfloat16)
    for c in range(NCORES):
        r = rel[c].reshape(total_sub, 128)          # [s, e]
        s_ix, e_ix = np.nonzero(r < 128)
        j_ix = r[s_ix, e_ix].astype(np.int64)
        indT[c, j_ix, s_ix * 128 + e_ix] = 1.0
    return isrc, drel, indT, inv, tile_of_sub, segs, total_sub


def _prep_inputs(inputs):
    x = np.asarray(inputs["x"], np.float32)
    ei = np.asarray(inputs["edge_index"])
    isrc, drel, indT, inv, tile_of_sub, segs, nsub = _schedule(ei)

    xdev = np.zeros((NCORES, PR, INP), np.float32)
    for c in range(NCORES):
        xdev[c, inv[c], :IN] = x[c * RPC:(c + 1) * RPC]
    # host-pre-transposed layer-0 lhsT: [128, TILES*2*128]
    xT = np.zeros((NCORES, 128, TILES * 2 * 128), ml_dtypes.bfloat16)
    for t in range(TILES):
        for kc in range(2):
            blk = xdev[:, t * 128:(t + 1) * 128, kc * 128:(kc + 1) * 128]
            xT[:, :, (t * 2 + kc) * 128:(t * 2 + kc + 1) * 128] = (
                blk.transpose(0, 2, 1).astype(ml_dtypes.bfloat16))

    def packw(W, a_s, a_d, d_in):
        W64 = np.asarray(W, np.float64)
        a_s = np.asarray(a_s, np.float64)
        a_d = np.asarray(a_d, np.float64)
        Wp = np.zeros((d_in, WA), np.float64)
        Wp[:W64.shape[0], :HC] = W64
        # folded attention halves: a_s[n,h] = sum_c h[n,h*C+c]*att_src[h,c]
        for h in range(H):
            Wp[:W64.shape[0], HC + h] = W64[:, h * C:(h + 1) * C] @ a_s[h]
            Wp[:W64.shape[0], HC + H + h] = W64[:, h * C:(h + 1) * C] @ a_d[h]
        return Wp.astype(np.float32).astype(ml_dtypes.bfloat16)

    rep = lambda v: np.broadcast_to(np.asarray(v, np.float32).reshape(1, -1), (128, v.size)).copy()
    fcw = np.asarray(inputs["fc_W"], np.float32)          # [1024, 6]
    fcw_sb = fcw.reshape(8, 128, OUT).transpose(1, 0, 2).reshape(128, 8 * OUT)
    fcw_sb = fcw_sb.astype(ml_dtypes.bfloat16)

    iota = np.broadcast_to(np.arange(128, dtype=np.float32), (128, 128)).copy()
    common = {
        "w0": packw(inputs["W0"], inputs["att_src0"], inputs["att_dst0"], INP),
        "w1": packw(inputs["W1"], inputs["att_src1"], inputs["att_dst1"], HC),
        "w2": packw(inputs["W2"], inputs["att_src2"], inputs["att_dst2"], HC),
        "fcw": fcw_sb,
        "fcb": rep(np.asarray(inputs["fc_b"], np.float32)),
        "ident": np.eye(128, dtype=ml_dtypes.bfloat16),
        "iota": iota.astype(ml_dtypes.bfloat16),
    }
    for l in range(3):
        common[f"brep{l}"] = rep(np.asarray(inputs[f"b{l}"], np.float32))

    has_bias = any(float(np.abs(np.asarray(inputs[f"b{l}"])).max()) > 0
                   for l in range(3))
    in_maps = []
    for c in range(NCORES):
        m = dict(common)
        m["xinT"] = xT[c]
        m["isrc"] = isrc[c]
        m["drel"] = drel[c]
        m["indT"] = indT[c]
        in_maps.append(m)
    return in_maps, inv, tile_of_sub, segs, nsub, has_bias


class _Ctx:
    """Shared emission state across the layer pipeline."""
    pass


def build_program(nc, tile_mod, mybir, tile_of_sub, segs, nsub, nlayers=3,
                  has_bias=True):
    """Emit the full 3-layer GAT program into `nc` (a Bacc) under TileContext."""
    from concourse.tile_rust import add_dep_helper
    f32 = mybir.dt.float32
    bf16 = mybir.dt.bfloat16
    i16 = mybir.dt.int16
    Alu = mybir.AluOpType
    Act = mybir.ActivationFunctionType

    nch = _nchunks()
    tpc = TILES // nch
    kmax = max(sum(kk for _, kk in tk) for _, tk, _ in segs)

    din = {
        "xinT": ((128, TILES * 2 * 128), bf16),
        "w0": ((INP, WA), bf16), "w1": ((HC, WA), bf16), "w2": ((HC, WA), bf16),
        "fcw": ((128, 8 * OUT), bf16), "fcb": ((128, OUT), f32),
        "ident": ((128, 128), bf16), "iota": ((128, 128), bf16),
        "isrc": ((128, nsub * 8), i16), "drel": ((128, nsub), bf16),
        "indT": ((128, nsub * 128), bf16),
    }
    for l in range(3):
        din[f"brep{l}"] = ((128, HC), f32)
    ins = {k: nc.dram_tensor(k, s, d, kind="ExternalInput").ap() for k, (s, d) in din.items()}
    probs_o = nc.dram_tensor("probs", (PR, OUT), f32, kind="ExternalOutput").ap()
    logits_o = nc.dram_tensor("logits", (PR, OUT), f32, kind="ExternalOutput").ap()

    # subchunk schedule
    first_of = {}
    last_of = {}
    for s, t in enumerate(tile_of_sub):
        t = int(t)
        first_of.setdefault(t, s)
        last_of[t] = s

    cx = _Ctx()
    cx.add_dep = add_dep_helper
    cx.mybir, cx.Alu, cx.Act = mybir, Alu, Act
    cx.f32, cx.bf16 = f32, bf16
    cx.nch, cx.tpc, cx.kmax, cx.nsub = nch, tpc, kmax, nsub
    cx.ins = ins
    cx.probs_o, cx.logits_o = probs_o, logits_o
    cx.first_of, cx.last_of, cx.tos = first_of, last_of, tile_of_sub
    cx.nlayers, cx.has_bias = nlayers, has_bias
    cx.brep = {}

    with tile_mod.TileContext(nc) as tc:
        with (
            tc.tile_pool(name="const", bufs=1) as cpool,
            tc.tile_pool(name="wpool", bufs=1) as wpool,
            tc.tile_pool(name="xtp", bufs=1) as xtp,
            tc.tile_pool(name="io", bufs=3) as iop,
            tc.tile_pool(name="gather", bufs=3) as gp,
            tc.tile_pool(name="msgp", bufs=4) as mp,
            tc.tile_pool(name="windp", bufs=3) as wp2,
            tc.tile_pool(name="zdp", bufs=3) as zp,
            tc.tile_pool(name="small", bufs=4) as sp,
            tc.tile_pool(name="fin", bufs=2) as fp,
            tc.tile_pool(name="pbig", bufs=2, space="PSUM") as pbig,
            tc.tile_pool(name="phd", bufs=3, space="PSUM") as phd,
            tc.tile_pool(name="ps", bufs=1, space="PSUM") as ps,
            tc.tile_pool(name="dram", bufs=1, space="DRAM") as dp,
        ):
            cx.wpool, cx.iop, cx.gp, cx.mp, cx.wp2 = wpool, iop, gp, mp, wp2
            cx.zp, cx.sp, cx.fp = zp, sp, fp
            cx.pbig, cx.phd, cx.dp = pbig, phd, dp

            cx.ident = cpool.tile([128, 128], bf16, name="ident_sb")
            nc.sync.dma_start(cx.ident[:, :], ins["ident"])
            cx.iota = cpool.tile([128, 128], bf16, name="iota_sb")
            nc.sync.dma_start(cx.iota[:, :], ins["iota"])
            cx.drel = cpool.tile([128, nsub], bf16, name="drel_sb")
            nc.sync.dma_start(cx.drel[:, :], ins["drel"])
            cx.isrc = cpool.tile([128, nsub * 8], i16, name="isrc_sb")
            nc.sync.dma_start(cx.isrc[:, :], ins["isrc"])
            cx.fcw = cpool.tile([128, 8 * OUT], bf16, name="fcw_sb")
            nc.sync.dma_start(cx.fcw[:, :], ins["fcw"])
            cx.fcb = cpool.tile([128, OUT], f32, name="fcb_sb")
            nc.sync.dma_start(cx.fcb[:, :], ins["fcb"])
            # resident next-layer lhsT (in-place: tile t's block is rewritten
            # by finalize(l) only after dense(l) of tile t consumed it).
            # Layer 0's host-pre-transposed lhsT is DMA'd straight into the
            # k-chunk-0/1 block slots.
            cx.xT_sb = xtp.tile([128, TILES * HC], bf16, name="xT_sb")
            for t in range(TILES):
                for kc in range(2):
                    nc.sync.dma_start(
                        cx.xT_sb[:, t * HC + kc * 128:t * HC + (kc + 1) * 128],
                        ins["xinT"][:, (t * 2 + kc) * 128:(t * 2 + kc + 1) * 128])

            cx.h_local = dp.tile([PR, HAUG], bf16, name="h_aug_local")
            # single-bank PSUM scratch, single-write slices only:
            #   [0:32] zd (pre-pass); [32+8*s4 : 40+8*s4] edge [w|alpha]
            #   per-subchunk partials (accumulated into SBUF walsum)
            cx.ps_bank = ps.tile([128, 512], f32, name="ps_bank")
            # SBUF accumulator for per-dst [w|alpha] sums, 2 tile-parity slots
            cx.walsum = cpool.tile([128, 16], f32, name="walsum_sb")

            cx.prev_ags = [None] * nch   # AG chunk insts of layer l-1
            cx.cur_ags = [None] * nch    # AG chunk insts of layer l
            cx.chunk_dmas = []
            cx.qsem = [nc.alloc_semaphore(f"gq{q}") for q in range(4)]
            cx.qcnt = [0, 0, 0, 0]   # completed gathers per queue (all layers)
            cx.wsb = {}
            cx.adloc = {}
            cx.hfull = {}

            # layer-0 weights + dense phase (all tiles), AG chunks fire inline
            _emit_wsb(nc, cx, 0)
            for t in range(TILES):
                _emit_dense_tile(nc, cx, 0, t)

            for l in range(nlayers):
                cx.ags = cx.cur_ags
                cx.cur_ags = [None] * nch
                if l + 1 < nlayers:
                    _emit_wsb(nc, cx, l + 1)
                # zd pre-pass (no AG dependency; fills the AG wait)
                for si, seg in enumerate(segs):
                    _emit_prepass_seg(nc, cx, l, si, seg, cx.zdsb[l])
                # edge phase; finalize(t) emits dense(l+1, t) + AG chunks.
                # Gathers use prepare_only so the serial Q7 descriptor
                # generation of the first pipeline-depth groups runs during
                # the AllGather wait; the cheap trigger fires the DMA once
                # the AG lands. Explicit-count protocol: a gpsimd
                # wait_ge(dma_sem) is the consumers' completion anchor.
                cx.agg = {}
                cx.preps = {}
                nseg = len(segs)
                for si in range(min(4, nseg)):
                    _emit_gather_prep(nc, cx, l, si, segs[si])
                for si in range(nseg + 1):
                    if si < nseg:
                        q = si % 4
                        trig = nc.gpsimd.trigger_dma(count=1, queue_num=q)
                        for agk in cx.ags:
                            cx.add_dep(trig.ins, agk.ins,
                                       reason="gather DMA after AG")
                    if si >= 1:
                        g = si - 1
                        qg = g % 4
                        cx.qcnt[qg] += 1
                        w = nc.gpsimd.wait_ge(cx.qsem[qg], 16 * cx.qcnt[qg])
                        _emit_edge_seg(nc, cx, l, g, segs[g], cx.zdsb[l], w)
                        if g + 4 < nseg:
                            _emit_gather_prep(nc, cx, l, g + 4, segs[g + 4])
    nc.compile()
    return nc


def _emit_wsb(nc, cx, l):
    kch = 2 if l == 0 else 8
    wsb = cx.wpool.tile([128, kch * WA], cx.bf16, tag="wsb", name=f"w_sb{l}")
    for kc in range(kch):
        nc.sync.dma_start(wsb[:, kc * WA:(kc + 1) * WA],
                          cx.ins[f"w{l}"][kc * 128:(kc + 1) * 128, :])
    cx.wsb[l] = wsb
    if cx.has_bias:
        brep = cx.wpool.tile([128, HC], cx.f32, tag="brep", name=f"brep_sb{l}")
        nc.sync.dma_start(brep[:, :], cx.ins[f"brep{l}"])
        cx.brep = getattr(cx, "brep", {})
        cx.brep[l] = brep
    cx.adloc[l] = cx.sp.tile([128, TILES * 4], cx.bf16, tag="adloc",
                             name=f"adloc{l}")
    cx.zdsb = getattr(cx, "zdsb", {})
    cx.zdsb[l] = cx.zp.tile([128, cx.nsub * 4], cx.bf16, tag="zdsb",
                            name=f"zdsb{l}")
    # shared collective outputs, one per (layer, src half): Shared DRAM
    # tiles are single-writer, so each half AG gets its own
    hpr = PR // cx.nch
    for q in range(cx.nch):
        cx.hfull[(l, q)] = cx.dp.tile(
            [NCORES * hpr, HAUG], cx.bf16, name=f"h_aug_full{l}_{q}",
            tag=f"hfull{l}_{q}", addr_space="Shared")


def _emit_dense_tile(nc, cx, l, t):
    """Dense phase for one 128-row tile of layer l; fires the AG chunk when
    this tile completes a chunk boundary."""
    Act, f32, bf16 = cx.Act, cx.f32, cx.bf16
    Alu = cx.Alu
    kch = 2 if l == 0 else 8
    wsb = cx.wsb[l]
    r0 = t * 128
    rowb = cx.iop.tile([128, HAUG], bf16, tag="rowb", name=f"rb{l}_{t}")
    rowbF = rowb[:, :].bitcast(f32)      # [128, 576]

    def xTb(kc):
        return cx.xT_sb[:, t * HC + kc * 128:t * HC + (kc + 1) * 128]

    # h in two 512-col bank generations, then a_s|a_d as a third
    for sl in range(2):
        ph = cx.phd.tile([128, 512], f32, tag="phd", name=f"ph{l}_{t}_{sl}")
        for kc in range(kch):
            nc.tensor.matmul(
                ph[:, :], lhsT=xTb(kc),
                rhs=wsb[:, kc * WA + sl * 512: kc * WA + (sl + 1) * 512],
                start=(kc == 0), stop=(kc == kch - 1))
        nc.scalar.activation(rowb[:, sl * 512:(sl + 1) * 512], ph[:, :], Act.Copy)
    pa = cx.phd.tile([128, 512], f32, tag="phd", name=f"pa{l}_{t}")
    for kc in range(kch):
        nc.tensor.matmul(
            pa[:, 0:8], lhsT=xTb(kc), rhs=wsb[:, kc * WA + HC: kc * WA + WA],
            start=(kc == 0), stop=(kc == kch - 1))
    # raw fp32 a_s into the row (bit-exact via bitcast view)
    nc.vector.tensor_copy(rowbF[:, 512:516], pa[:, 0:4])
    # a_d -> resident bf16 table
    nc.scalar.activation(cx.adloc[l][:, t * 4:(t + 1) * 4], pa[:, 4:8], Act.Copy)
    nc.vector.memset(rowb[:, HC + 8:HAUG], 0.0)
    rbd = nc.sync.dma_start(cx.h_local[r0:r0 + 128, :], rowb[:, :])
    ck = t // cx.tpc
    cx.chunk_dmas.append(rbd)
    if cx.prev_ags[ck] is not None:
        cx.add_dep(rbd.ins, cx.prev_ags[ck].ins,
                   reason="h_local WAR vs prev AG chunk")
    if t % cx.tpc == cx.tpc - 1:
        rows = cx.tpc * 128
        ag = nc.gpsimd.collective_compute(
            "AllGather", cx.Alu.bypass,
            replica_groups=[list(range(NCORES))],
            ins=[cx.h_local[ck * rows:(ck + 1) * rows, :].opt()],
            outs=[cx.hfull[(l, ck)][:, :].opt()],
        )
        for rbd2 in cx.chunk_dmas:
            cx.add_dep(ag.ins, rbd2.ins, reason="AG after h_local chunk writes")
        cx.chunk_dmas = []
        cx.cur_ags[ck] = ag
        cx.prev_ags[ck] = ag


def _emit_prepass_seg(nc, cx, l, si, seg, zdsb):
    """Per-edge a_d via host-shipped transposed one-hot + small matmuls."""
    Act, Alu, f32, bf16 = cx.Act, cx.Alu, cx.f32, cx.bf16
    q, tk, b0 = seg
    nsb = sum(kk for _, kk in tk)
    itg = cx.zp.tile([128, cx.kmax * 128], bf16, tag="itg", name=f"itg{l}_{si}")
    nc.sync.dma_start(itg[:, 0:nsb * 128],
                      cx.ins["indT"][:, b0 * 128:(b0 + nsb) * 128])
    # double-buffered zd slice (group parity) so group si+1's matmuls don't
    # serialize behind group si's zdsb copy
    zb = 128 + (si % 2) * 64
    zd = cx.ps_bank[:, zb:zb + nsb * 4]
    for s4 in range(nsb):
        td = int(cx.tos[b0 + s4])
        nc.tensor.matmul(zd[:, s4 * 4:(s4 + 1) * 4],
                         lhsT=itg[:, s4 * 128:(s4 + 1) * 128],
                         rhs=cx.adloc[l][:, td * 4:(td + 1) * 4],
                         start=True, stop=True)
    nc.scalar.activation(zdsb[:, b0 * 4:(b0 + nsb) * 4], zd, Act.Copy)


def _emit_gather_prep(nc, cx, l, si, seg):
    """SWDGE descriptor generation for group si's gather (prepare_only)."""
    q, tk, sb0 = seg
    nsb = sum(kk for _, kk in tk)
    ic = cx.isrc[:, sb0 * 8:(sb0 + nsb) * 8]
    hr = cx.mp.tile([128, cx.kmax * HAUG], cx.bf16, tag="hr", name=f"hr{l}_{si}")
    nc.gpsimd.dma_gather(
        hr[:, 0:nsb * HAUG].rearrange("p (a b) -> p a b", b=HAUG),
        cx.hfull[(l, q)][:, 0:HAUG], ic, nsb * 128, nsb * 128, HAUG,
        elem_step=HAUG, single_packet=False, queue_num=si % 4,
        prepare_only=True, sem=cx.qsem[si % 4])
    cx.preps[si] = hr


def _emit_edge_seg(nc, cx, l, si, seg, zdsb, w):
    Act, Alu, f32, bf16 = cx.Act, cx.Alu, cx.f32, cx.bf16
    q, tk, sb0 = seg
    nsb = sum(kk for _, kk in tk)
    ifg = cx.gp.tile([128, cx.kmax * 128], bf16, tag="ifg", name=f"if{l}_{si}")
    nc.vector.tensor_tensor(
        ifg[:, 0:nsb * 128].rearrange("p (s j) -> p s j", j=128),
        cx.iota[:, :].unsqueeze(1).broadcast_to((128, nsb, 128)),
        cx.drel[:, sb0:sb0 + nsb]
            .unsqueeze(2).broadcast_to((128, nsb, 128)),
        Alu.is_equal)
    hr = cx.preps.pop(si)

    hrF = hr[:, :].bitcast(f32)    # [128, kmax*576]
    z = cx.sp.tile([128, cx.kmax * 4], f32, tag="z", name=f"z{l}_{si}")
    zi = nc.vector.tensor_tensor(
        z[:, 0:nsb * 4].rearrange("p (a b) -> p a b", b=4),
        hrF.rearrange("p (s c) -> p s c", c=576)[:, 0:nsb, 512:516],
        zdsb[:, sb0 * 4:(sb0 + nsb) * 4].rearrange("p (a b) -> p a b", b=4),
        Alu.add)
    # hr readers must anchor on DMA completion (the wait_ge), not on the
    # prep's desc-gen tick
    cx.add_dep(zi.ins, w.ins, reason="hr RAW on gather DMA completion")
    # walpha bf16 [128, nsb*8]: per s4 [w(4) | alpha(4)]
    wal = cx.sp.tile([128, cx.kmax * 8], bf16, tag="wal", name=f"wa{l}_{si}")
    wal3 = wal[:, 0:nsb * 8].rearrange("p (s x) -> p s x", x=8)
    nc.vector.scalar_tensor_tensor(
        wal3[:, :, 4:8],
        z[:, 0:nsb * 4].rearrange("p (s h) -> p s h", h=H),
        NEG, z[:, 0:nsb * 4].rearrange("p (s h) -> p s h", h=H),
        Alu.mult, Alu.max)
    nc.scalar.activation(wal3[:, :, 0:4], wal3[:, :, 4:8], Act.Exp)
    # f32 w for the scalar-engine per-partition scale (second exp, same
    # bf16 alpha input -> bf16(w_f32) == the bf16 w above)
    wf = cx.sp.tile([128, cx.kmax * 4], f32, tag="wf", name=f"wf{l}_{si}")
    nc.scalar.activation(
        wf[:, 0:nsb * 4].rearrange("p (s h) -> p s h", h=H),
        wal3[:, :, 4:8], Act.Exp)

    # per-segment weighted indicators: head 0 on ScalarE (per-partition
    # scale), heads 1-3 in one batched DVE op
    wind = cx.wp2.tile([128, cx.kmax * H * 128], bf16, tag="wind",
                       name=f"wi{l}_{si}")
    wind4 = wind[:, 0:nsb * H * 128].rearrange("p (s h j) -> p s h j",
                                               h=H, j=128)
    nc.vector.tensor_tensor(
        wind4[:, :, 1:4, :],
        ifg[:, 0:nsb * 128].rearrange("p (s j) -> p s j", j=128)
            .unsqueeze(2).broadcast_to((128, nsb, 3, 128)),
        wal3[:, :, 1:4].unsqueeze(3).broadcast_to((128, nsb, 3, 128)),
        Alu.mult)
    for s4 in range(nsb):
        nc.scalar.activation(
            wind[:, s4 * H * 128:s4 * H * 128 + 128],
            ifg[:, s4 * 128:(s4 + 1) * 128], Act.Copy,
            scale=wf[:, s4 * 4:s4 * 4 + 1])

    # per-tile runs within the segment (tk gives tile order + counts)
    s4a = 0
    for ri, (t, kk) in enumerate(tk):
        s4b = s4a + kk - 1
        if t not in cx.agg:
            cx.agg[t] = cx.pbig.tile([128, HC], f32, tag="pbig",
                                     name=f"agg{l}_{t}")
        P = cx.agg[t]
        Pa = cx.walsum[:, (t % 2) * 8:(t % 2) * 8 + 8]
        fi = cx.first_of[t] == sb0 + s4a
        la = cx.last_of[t] == sb0 + s4b
        # [w|alpha] partial for the whole run (own PSUM accumulation in the
        # scratch bank), then one DVE accumulate into the walsum parity slot
        pp = cx.ps_bank[:, 64 + (ri % 2) * 8:72 + (ri % 2) * 8]
        for s4 in range(s4a, s4b + 1):
            b0 = s4 * HAUG
            # 2 heads share a 2KB PSUM zero-region (bank): only the first
            # matmul per bank may carry start, only the last may carry stop.
            for hd in range(H):
                mi = nc.tensor.matmul(
                    P[:, hd * C:(hd + 1) * C],
                    lhsT=wind[:, s4 * H * 128 + hd * 128:
                              s4 * H * 128 + (hd + 1) * 128],
                    rhs=hr[:, b0 + hd * C:b0 + (hd + 1) * C],
                    start=fi and s4 == s4a and hd % 2 == 0,
                    stop=la and s4 == s4b and hd % 2 == 1)
                cx.add_dep(mi.ins, w.ins,
                           reason="hr RAW on gather DMA completion")
            nc.tensor.matmul(pp, lhsT=ifg[:, s4 * 128:(s4 + 1) * 128],
                             rhs=wal[:, s4 * 8:(s4 + 1) * 8],
                             start=s4 == s4a, stop=s4 == s4b)
        if fi:
            nc.vector.tensor_copy(Pa, pp)
        else:
            nc.vector.tensor_tensor(Pa, Pa, pp, Alu.add)
        if la:
            _finalize(nc, cx, l, t, P, Pa)
            del cx.agg[t]
            if l + 1 < cx.nlayers:
                _emit_dense_tile(nc, cx, l + 1, t)
        s4a = s4b + 1


def _finalize(nc, cx, l, t, P, Pa):
    Alu, Act = cx.Alu, cx.Act
    f32, bf16 = cx.f32, cx.bf16
    sp, fp = cx.sp, cx.fp
    r0 = t * 128
    t1 = sp.tile([128, 4], f32, tag="t1", name=f"t1{l}_{t}")
    nc.scalar.activation(t1[:, :], Pa[:, 4:8], Act.Exp, scale=-1.0)
    ts = sp.tile([128, 4], f32, tag="ts", name=f"ts{l}_{t}")
    nc.vector.tensor_tensor(ts[:, :], t1[:, :], Pa[:, 0:4], Alu.mult)
    nc.vector.tensor_scalar_add(ts[:, :], ts[:, :], 1e-16)
    rc = sp.tile([128, 4], f32, tag="rc", name=f"rc{l}_{t}")
    nc.vector.reciprocal(rc[:, :], ts[:, :])
    cf = sp.tile([128, 4], f32, tag="cf", name=f"cf{l}_{t}")
    nc.vector.tensor_tensor(cf[:, :], t1[:, :], rc[:, :], Alu.mult)
    outb = fp.tile([128, HC], f32, tag="outb", name=f"ob{l}_{t}")
    nc.vector.tensor_tensor(
        outb[:, :].rearrange("p (h c) -> p h c", c=C),
        P[:, 0:HC].rearrange("p (h c) -> p h c", c=C),
        cf[:, :].unsqueeze(2).broadcast_to((128, H, C)), Alu.mult)
    if cx.has_bias:
        nc.vector.tensor_tensor(outb[:, :], outb[:, :], cx.brep[l][:, :], Alu.add)
    relu = fp.tile([128, HC], bf16, tag="relu", name=f"rl{l}_{t}")
    nc.scalar.activation(relu[:, :], outb[:, :], Act.Relu)
    def pe_transpose_to(dst_of_kc, nm):
        # PE-transpose via the phd scratch banks; psum->sbuf copies split
        # across ScalarE and DVE
        for kc in range(8):
            pt = cx.phd.tile([128, 512], cx.f32, tag="phd", name=f"{nm}_{kc}")
            ptw = pt[:, 0:64].bitcast(bf16)
            nc.tensor.transpose(ptw, relu[:, kc * 128:(kc + 1) * 128],
                                cx.ident[:, :])
            if kc % 2 == 0:
                nc.scalar.activation(dst_of_kc(kc), ptw, Act.Copy)
            else:
                nc.vector.tensor_copy(dst_of_kc(kc), ptw)

    if l < cx.nlayers - 1:
        # PE-transpose into the resident next-layer lhsT (no DRAM round trip)
        pe_transpose_to(
            lambda kc: cx.xT_sb[:, t * HC + kc * 128:t * HC + (kc + 1) * 128],
            f"xt{l}_{t}")
        return
    # final layer: fc head + row softmax
    hT = fp.tile([128, HC], bf16, tag="hT", name=f"hT{t}")
    pe_transpose_to(lambda kc: hT[:, kc * 128:(kc + 1) * 128], f"ft{t}")
    plt = cx.phd.tile([128, 512], f32, tag="phd", name=f"pl{t}")
    pl = plt[:, 0:16]
    for kc in range(8):
        nc.tensor.matmul(pl[:, 0:OUT], lhsT=hT[:, kc * 128:(kc + 1) * 128],
                         rhs=cx.fcw[:, kc * OUT:(kc + 1) * OUT],
                         start=(kc == 0), stop=(kc == 7))
    lg = sp.tile([128, OUT], f32, tag="lg", name=f"lg{t}")
    nc.vector.tensor_tensor(lg[:, :], pl[:, 0:OUT], cx.fcb[:, :], Alu.add)
    nc.sync.dma_start(cx.logits_o[r0:r0 + 128, :], lg[:, :])
    mx = sp.tile([128, 1], f32, tag="mx", name=f"mx{t}")
    nc.vector.tensor_reduce(mx[:, :], lg[:, :], cx.mybir.AxisListType.X, Alu.max)
    l2 = sp.tile([128, OUT], f32, tag="l2", name=f"l2{t}")
    nc.vector.tensor_scalar_sub(l2[:, :], lg[:, :], mx[:, 0:1])
    ex = sp.tile([128, OUT], f32, tag="ex", name=f"ex{t}")
    se = sp.tile([128, 1], f32, tag="se", name=f"se{t}")
    nc.scalar.activation(ex[:, :], l2[:, :], Act.Exp, accum_out=se[:, :])
    rs = sp.tile([128, 1], f32, tag="rs", name=f"rs{t}")
    nc.vector.reciprocal(rs[:, :], se[:, :])
    pb = sp.tile([128, OUT], f32, tag="pb", name=f"pb{t}")
    nc.vector.tensor_scalar_mul(pb[:, :], ex[:, :], rs[:, 0:1])
    nc.sync.dma_start(cx.probs_o[r0:r0 + 128, :], pb[:, :])


_CACHE = {}


def _build_and_run(inputs, trace=False):
    from concourse import bacc, tile, mybir
    from concourse.bass_utils import run_bass_kernel_spmd

    in_maps, inv, tile_of_sub, segs, nsub, has_bias = _prep_inputs(inputs)
    key = (repr(segs), nsub, has_bias, tuple(int(t) for t in tile_of_sub))
    if key not in _CACHE:
        nc = bacc.Bacc("TRN2", target_bir_lowering=False, debug=False,
                       enable_asserts=False, num_devices=NCORES,
                       num_swdge_queues=4)
        build_program(nc, tile, mybir, tile_of_sub, segs, nsub,
                      has_bias=has_bias)
        _CACHE[key] = nc
    nc = _CACHE[key]
    res = run_bass_kernel_spmd(nc, in_maps, list(range(NCORES)), trace=trace)
    probs = np.empty((N, OUT), np.float32)
    logits = np.empty((N, OUT), np.float32)
    for c in range(NCORES):
        probs[c * RPC:(c + 1) * RPC] = res.results[c]["probs"][inv[c]]
        logits[c * RPC:(c + 1) * RPC] = res.results[c]["logits"][inv[c]]
    return (probs, logits), res


def kernel(**inputs):
    (probs, logits), _ = _build_and_run(inputs, trace=False)
    return probs, logits


# revision 79
# speedup vs baseline: 1.0875x; 1.0875x over previous
"""3-layer GAT (GATConv x3 + linear head + softmax) on 8 Trainium2 NeuronCores.

Strategy: nodes partitioned into 8 contiguous blocks (2500 real + 60 pad rows
per core -> 2560 = 20 tiles of 128). Edges assigned to the core owning their
dst node, sorted by dst tile. Per layer:
  1. dense phase (per 128-row tile): h_aug = x @ W' where W' = [W | W@att_src |
     W@att_dst] (attention halves folded into the matmul on host, fp64). lhsT
     for layer 0 is host-pre-transposed; for layers 1-2 it is PE-transposed
     from the previous layer's relu output and kept resident in SBUF (no DRAM
     round trip, no DMA transposes). Layer l+1's dense tile t is EMITTED inline
     right after finalize(l, t) so it truly overlaps layer l's edge phase in
     every engine's (in-order) instruction stream. h_aug rows (bf16 h | raw
     fp32 a_s via bitcast) go to h_local DRAM; a_d to a resident bf16 table.
  2. one AllGather per layer into a Shared h_full tile, fired right after the
     last dense tile. (Chunked/half AGs were tried and are net losses: the
     collective executes on -- and blocks -- the in-order gpsimd queue that
     also issues the gathers.)
  3. zd pre-pass (overlaps the edge tail / AG wait): per group, DMA the
     host-shipped transposed one-hot indicator and matmul it with adloc to
     give every edge's a_d -> zdsb. No AG dependency; the zd PSUM slice is
     double-buffered by group parity so groups pipeline.
  4. edge phase per 1024-edge group (SWDGE dma_gather, 4 queues, 4-deep hr
     buffering -- the Q7 descriptor generation at ~7.6ns/edge is the edge
     bottleneck, so pipeline depth matters): one gather pulls src rows
     (2304B/edge); z = a_s + a_d, alpha = leaky(z), w = exp(alpha); the
     forward one-hot ifg is built on-device (iota is_equal dstrel); per-head
     weighted indicators (wind = w * ifg) built with head 0 on ScalarE
     (per-partition-scale activation) and heads 1-3 in one batched DVE op;
     per 128-edge subchunk 4x256-col matmuls scatter-add w*h, and per-tile
     runs accumulate [w | alpha] partials in a scratch PSUM bank that DVE
     folds into an SBUF accumulator.
  5. tile finalize: out = (num * exp(-m)/(exp(-m)*s + 1e-16)) + b, relu; then
     PE-transpose into the next layer's resident lhsT. The exp(-m) factor
     reproduces the reference-as-executed softmax shift exactly (segment_max
     lowers to segment_sum on this platform).
Final layer fuses the fc head + row softmax; outputs concatenated on host.
PSUM layout (8 banks): agg h 2x2 | dense ph 3x1 (512-col generations, also
transposes + fc) | scratch bank (zd x2 parity slices + [w|alpha] partials).
"""
import sys

sys.path.insert(0, "/opt/trn_rl_repo")

import ml_dtypes
import numpy as np

N = 20000
E = 320000
IN = 131
INP = 256          # IN padded to 2 k-chunks
H = 4
C = 256
HC = 1024
WA = 1032          # W' columns: 1024 h | 4 a_s | 4 a_d
OUT = 6
NEG = 0.2
NCORES = 8
RPC = 2500         # real rows per core
PR = 2560          # padded rows per core (20 tiles of 128)
TILES = PR // 128
HAUG = 1152        # bf16 h_aug row: 1024 h | 8 (4 fp32 a_s) | 8 spare | pad
CHUNKS = 1         # src halves == AllGather chunks. Collectives execute on
                   # (and block) the in-order gpsimd queue that also issues
                   # the gathers, so a mid-edge half-AG stalls the remaining
                   # edge phase; a single AG per layer is net faster.
SEGCAP = 8         # subchunks (x128 edges) per gather group


def _nchunks():
    return CHUNKS if TILES % CHUNKS == 0 else (2 if TILES % 2 == 0 else 1)


def _schedule(edge_index: np.ndarray):
    """Partition + sort edges; build per-core device arrays and the shared
    compile-time segment schedule: a tile-major subchunk stream cut into
    SEGCAP-subchunk gather groups that may span dst-tile boundaries (at most
    2 PSUM agg tiles are ever open)."""
    nch = _nchunks()
    hpr = PR // nch             # rows per src half
    WT = 1                      # dst tiles per window

    src_g = np.concatenate([edge_index[0], np.arange(N, dtype=np.int64)])
    dst_g = np.concatenate([edge_index[1], np.arange(N, dtype=np.int64)])
    dst_l = dst_g % RPC                   # local dst row in [0, RPC)
    core = dst_g // RPC

    # Per-core row permutation: bin-pack nodes into tiles balanced by
    # incoming-edge count, so the shared (max-across-cores) subchunk schedule
    # carries less padding. inv[c][orig_local] = permuted local row.
    inv = np.zeros((NCORES, RPC), np.int64)
    for c in range(NCORES):
        deg = np.bincount(dst_l[core == c], minlength=RPC)
        order = np.argsort(-deg, kind="stable")
        tsum = np.zeros(TILES, np.int64)
        tfill = np.zeros(TILES, np.int64)
        for j in order:
            open_t = np.flatnonzero(tfill < 128)
            tt = open_t[np.argmin(tsum[open_t])]
            inv[c, j] = tt * 128 + tfill[tt]
            tfill[tt] += 1
            tsum[tt] += deg[j]

    src_c = src_g // RPC
    src_l = inv[src_c, src_g % RPC]
    half = src_l // hpr
    src_d = src_c * hpr + (src_l % hpr)   # row id within the half tensor

    per_core = []
    counts = np.zeros((NCORES, TILES, nch), np.int64)
    for c in range(NCORES):
        sel = core == c
        s = src_d[sel]
        q = half[sel]
        dl = inv[c, dst_l[sel]]
        t = dl // 128
        order = np.lexsort((dl, q, t))
        s, q, dl, t = s[order], q[order], dl[order], t[order]
        np.add.at(counts[c], (t, q), 1)
        per_core.append((s, q, dl, t))

    k = np.ceil(counts.max(axis=0) / 128).astype(np.int64)   # [TILES, nch]

    # stream: per window of WT tiles, per half, the (tile, half) buckets
    segs = []       # (q, [(tile, k_t), ...], base_sub)
    tile_of_sub = []
    base = {}       # (t, q) -> slot base
    # tile-major subchunk stream (each tile >=1 subchunk), padded to a
    # multiple of SEGCAP; cut into SEGCAP-subchunk gather groups that may
    # span dst-tile boundaries (the per-tile PSUM runs handle that)
    assert nch == 1
    kt = np.maximum(1, k[:, 0])
    kt[TILES - 1] += (-int(kt.sum())) % SEGCAP
    for t in range(TILES):
        base[(t, 0)] = len(tile_of_sub) * 128
        tile_of_sub.extend([t] * int(kt[t]))
    for g0 in range(0, len(tile_of_sub), SEGCAP):
        chunk = tile_of_sub[g0:g0 + SEGCAP]
        tk = []
        for t in chunk:
            if tk and tk[-1][0] == t:
                tk[-1][1] += 1
            else:
                tk.append([t, 1])
        segs.append((0, [tuple(x) for x in tk], g0))
    total_sub = len(tile_of_sub)
    tile_of_sub = np.asarray(tile_of_sub)

    srcA = np.zeros((NCORES, total_sub * 128), np.int16)
    rel = np.full((NCORES, total_sub * 128), 200.0, np.float32)
    for c in range(NCORES):
        s, q, dl, t = per_core[c]
        for (tt, qq), b in base.items():
            m = (t == tt) & (q == qq)
            n = int(m.sum())
            srcA[c, b:b + n] = s[m].astype(np.int16)
            rel[c, b:b + n] = (dl[m] - tt * 128).astype(np.float32)

    # per-segment 16-partition wrap (8x replicated), concatenated columns:
    # segment at base_sub owns isrc cols [base_sub*8, (base_sub+nsb)*8)
    isrc = np.zeros((NCORES, 128, total_sub * 8), np.int16)
    for q, tk, base_sub in segs:
        nsb = sum(kk for _, kk in tk)
        n = nsb * 128
        b = base_sub * 128
        a = srcA[:, b:b + n]
        wv = a.reshape(NCORES, n // 16, 16).transpose(0, 2, 1)
        isrc[:, :, base_sub * 8:(base_sub + nsb) * 8] = np.tile(wv, (1, 8, 1))

    # dstrel plane [128, nsub]: [p, s] = rel dst of edge s*128+p (bf16-exact)
    drel = rel.reshape(NCORES, total_sub, 128).transpose(0, 2, 1)
    drel = drel.astype(ml_dtypes.bfloat16).copy()
    # transposed one-hot indicator for the zd pre-pass:
    # indT[j, s*128+e] = 1 iff edge (s,e)'s relative dst row == j
    indT = np.zeros((NCORES, 128, total_sub * 128), ml_dtypes.bfloat16)
    for c in range(NCORES):
        r = rel[c].reshape(total_sub, 128)          # [s, e]
        s_ix, e_ix = np.nonzero(r < 128)
        j_ix = r[s_ix, e_ix].astype(np.int64)
        indT[c, j_ix, s_ix * 128 + e_ix] = 1.0
    return isrc, drel, indT, inv, tile_of_sub, segs, total_sub


def _prep_inputs(inputs):
    x = np.asarray(inputs["x"], np.float32)
    ei = np.asarray(inputs["edge_index"])
    isrc, drel, indT, inv, tile_of_sub, segs, nsub = _schedule(ei)

    xdev = np.zeros((NCORES, PR, INP), np.float32)
    for c in range(NCORES):
        xdev[c, inv[c], :IN] = x[c * RPC:(c + 1) * RPC]
    # host-pre-transposed layer-0 lhsT: [128, TILES*2*128]
    xT = np.zeros((NCORES, 128, TILES * 2 * 128), ml_dtypes.bfloat16)
    for t in range(TILES):
        for kc in range(2):
            blk = xdev[:, t * 128:(t + 1) * 128, kc * 128:(kc + 1) * 128]
            xT[:, :, (t * 2 + kc) * 128:(t * 2 + kc + 1) * 128] = (
                blk.transpose(0, 2, 1).astype(ml_dtypes.bfloat16))

    def packw(W, a_s, a_d, d_in):
        W64 = np.asarray(W, np.float64)
        a_s = np.asarray(a_s, np.float64)
        a_d = np.asarray(a_d, np.float64)
        Wp = np.zeros((d_in, WA), np.float64)
        Wp[:W64.shape[0], :HC] = W64
        # folded attention halves: a_s[n,h] = sum_c h[n,h*C+c]*att_src[h,c]
        for h in range(H):
            Wp[:W64.shape[0], HC + h] = W64[:, h * C:(h + 1) * C] @ a_s[h]
            Wp[:W64.shape[0], HC + H + h] = W64[:, h * C:(h + 1) * C] @ a_d[h]
        return Wp.astype(np.float32).astype(ml_dtypes.bfloat16)

    rep = lambda v: np.broadcast_to(np.asarray(v, np.float32).reshape(1, -1), (128, v.size)).copy()
    fcw = np.asarray(inputs["fc_W"], np.float32)          # [1024, 6]
    fcw_sb = fcw.reshape(8, 128, OUT).transpose(1, 0, 2).reshape(128, 8 * OUT)
    fcw_sb = fcw_sb.astype(ml_dtypes.bfloat16)

    iota = np.broadcast_to(np.arange(128, dtype=np.float32), (128, 128)).copy()
    common = {
        "w0": packw(inputs["W0"], inputs["att_src0"], inputs["att_dst0"], INP),
        "w1": packw(inputs["W1"], inputs["att_src1"], inputs["att_dst1"], HC),
        "w2": packw(inputs["W2"], inputs["att_src2"], inputs["att_dst2"], HC),
        "fcw": fcw_sb,
        "fcb": rep(np.asarray(inputs["fc_b"], np.float32)),
        "ident": np.eye(128, dtype=ml_dtypes.bfloat16),
        "iota": iota.astype(ml_dtypes.bfloat16),
    }
    for l in range(3):
        common[f"brep{l}"] = rep(np.asarray(inputs[f"b{l}"], np.float32))

    has_bias = any(float(np.abs(np.asarray(inputs[f"b{l}"])).max()) > 0
                   for l in range(3))
    in_maps = []
    for c in range(NCORES):
        m = dict(common)
        m["xinT"] = xT[c]
        m["isrc"] = isrc[c]
        m["drel"] = drel[c]
        m["indT"] = indT[c]
        in_maps.append(m)
    return in_maps, inv, tile_of_sub, segs, nsub, has_bias


class _Ctx:
    """Shared emission state across the layer pipeline."""
    pass


def build_program(nc, tile_mod, mybir, tile_of_sub, segs, nsub, nlayers=3,
                  has_bias=True):
    """Emit the full 3-layer GAT program into `nc` (a Bacc) under TileContext."""
    from concourse.tile_rust import add_dep_helper
    f32 = mybir.dt.float32
    bf16 = mybir.dt.bfloat16
    i16 = mybir.dt.int16
    Alu = mybir.AluOpType
    Act = mybir.ActivationFunctionType

    nch = _nchunks()
    tpc = TILES // nch
    kmax = max(sum(kk for _, kk in tk) for _, tk, _ in segs)

    din = {
        "xinT": ((128, TILES * 2 * 128), bf16),
        "w0": ((INP, WA), bf16), "w1": ((HC, WA), bf16), "w2": ((HC, WA), bf16),
        "fcw": ((128, 8 * OUT), bf16), "fcb": ((128, OUT), f32),
        "ident": ((128, 128), bf16), "iota": ((128, 128), bf16),
        "isrc": ((128, nsub * 8), i16), "drel": ((128, nsub), bf16),
        "indT": ((128, nsub * 128), bf16),
    }
    for l in range(3):
        din[f"brep{l}"] = ((128, HC), f32)
    ins = {k: nc.dram_tensor(k, s, d, kind="ExternalInput").ap() for k, (s, d) in din.items()}
    probs_o = nc.dram_tensor("probs", (PR, OUT), f32, kind="ExternalOutput").ap()
    logits_o = nc.dram_tensor("logits", (PR, OUT), f32, kind="ExternalOutput").ap()

    # subchunk schedule
    first_of = {}
    last_of = {}
    for s, t in enumerate(tile_of_sub):
        t = int(t)
        first_of.setdefault(t, s)
        last_of[t] = s

    cx = _Ctx()
    cx.add_dep = add_dep_helper
    cx.mybir, cx.Alu, cx.Act = mybir, Alu, Act
    cx.f32, cx.bf16 = f32, bf16
    cx.nch, cx.tpc, cx.kmax, cx.nsub = nch, tpc, kmax, nsub
    cx.ins = ins
    cx.probs_o, cx.logits_o = probs_o, logits_o
    cx.first_of, cx.last_of, cx.tos = first_of, last_of, tile_of_sub
    cx.nlayers, cx.has_bias = nlayers, has_bias
    cx.brep = {}

    with tile_mod.TileContext(nc) as tc:
        with (
            tc.tile_pool(name="const", bufs=1) as cpool,
            tc.tile_pool(name="wpool", bufs=1) as wpool,
            tc.tile_pool(name="xtp", bufs=1) as xtp,
            tc.tile_pool(name="io", bufs=3) as iop,
            tc.tile_pool(name="gather", bufs=3) as gp,
            tc.tile_pool(name="msgp", bufs=4) as mp,
            tc.tile_pool(name="windp", bufs=3) as wp2,
            tc.tile_pool(name="zdp", bufs=3) as zp,
            tc.tile_pool(name="small", bufs=4) as sp,
            tc.tile_pool(name="fin", bufs=2) as fp,
            tc.tile_pool(name="pbig", bufs=2, space="PSUM") as pbig,
            tc.tile_pool(name="phd", bufs=3, space="PSUM") as phd,
            tc.tile_pool(name="ps", bufs=1, space="PSUM") as ps,
            tc.tile_pool(name="dram", bufs=1, space="DRAM") as dp,
        ):
            cx.wpool, cx.iop, cx.gp, cx.mp, cx.wp2 = wpool, iop, gp, mp, wp2
            cx.zp, cx.sp, cx.fp = zp, sp, fp
            cx.pbig, cx.phd, cx.dp = pbig, phd, dp

            cx.ident = cpool.tile([128, 128], bf16, name="ident_sb")
            nc.sync.dma_start(cx.ident[:, :], ins["ident"])
            cx.iota = cpool.tile([128, 128], bf16, name="iota_sb")
            nc.sync.dma_start(cx.iota[:, :], ins["iota"])
            cx.drel = cpool.tile([128, nsub], bf16, name="drel_sb")
            nc.sync.dma_start(cx.drel[:, :], ins["drel"])
            cx.isrc = cpool.tile([128, nsub * 8], i16, name="isrc_sb")
            nc.sync.dma_start(cx.isrc[:, :], ins["isrc"])
            cx.fcw = cpool.tile([128, 8 * OUT], bf16, name="fcw_sb")
            nc.sync.dma_start(cx.fcw[:, :], ins["fcw"])
            cx.fcb = cpool.tile([128, OUT], f32, name="fcb_sb")
            nc.sync.dma_start(cx.fcb[:, :], ins["fcb"])
            # resident next-layer lhsT (in-place: tile t's block is rewritten
            # by finalize(l) only after dense(l) of tile t consumed it).
            # Layer 0's host-pre-transposed lhsT is DMA'd straight into the
            # k-chunk-0/1 block slots.
            cx.xT_sb = xtp.tile([128, TILES * HC], bf16, name="xT_sb")
            for t in range(TILES):
                for kc in range(2):
                    nc.sync.dma_start(
                        cx.xT_sb[:, t * HC + kc * 128:t * HC + (kc + 1) * 128],
                        ins["xinT"][:, (t * 2 + kc) * 128:(t * 2 + kc + 1) * 128])

            cx.h_local = dp.tile([PR, HAUG], bf16, name="h_aug_local")
            # single-bank PSUM scratch, single-write slices only:
            #   [0:32] zd (pre-pass); [32+8*s4 : 40+8*s4] edge [w|alpha]
            #   per-subchunk partials (accumulated into SBUF walsum)
            cx.ps_bank = ps.tile([128, 512], f32, name="ps_bank")
            # SBUF accumulator for per-dst [w|alpha] sums, 2 tile-parity slots
            cx.walsum = cpool.tile([128, 16], f32, name="walsum_sb")

            cx.prev_ags = [None] * nch   # AG chunk insts of layer l-1
            cx.cur_ags = [None] * nch    # AG chunk insts of layer l
            cx.chunk_dmas = []
            cx.wsb = {}
            cx.adloc = {}
            cx.hfull = {}

            # layer-0 weights + dense phase (all tiles), AG chunks fire inline
            _emit_wsb(nc, cx, 0)
            for t in range(TILES):
                _emit_dense_tile(nc, cx, 0, t)

            for l in range(nlayers):
                cx.ags = cx.cur_ags
                cx.cur_ags = [None] * nch
                if l + 1 < nlayers:
                    _emit_wsb(nc, cx, l + 1)
                # zd pre-pass (no AG dependency; fills the AG wait)
                for si, seg in enumerate(segs):
                    _emit_prepass_seg(nc, cx, l, si, seg, cx.zdsb[l])
                # edge phase; finalize(t) emits dense(l+1, t) + AG chunks
                cx.agg = {}
                for si, seg in enumerate(segs):
                    _emit_edge_seg(nc, cx, l, si, seg, cx.zdsb[l])
    nc.compile()
    return nc


def _emit_wsb(nc, cx, l):
    kch = 2 if l == 0 else 8
    wsb = cx.wpool.tile([128, kch * WA], cx.bf16, tag="wsb", name=f"w_sb{l}")
    for kc in range(kch):
        nc.sync.dma_start(wsb[:, kc * WA:(kc + 1) * WA],
                          cx.ins[f"w{l}"][kc * 128:(kc + 1) * 128, :])
    cx.wsb[l] = wsb
    if cx.has_bias:
        brep = cx.wpool.tile([128, HC], cx.f32, tag="brep", name=f"brep_sb{l}")
        nc.sync.dma_start(brep[:, :], cx.ins[f"brep{l}"])
        cx.brep = getattr(cx, "brep", {})
        cx.brep[l] = brep
    cx.adloc[l] = cx.sp.tile([128, TILES * 4], cx.bf16, tag="adloc",
                             name=f"adloc{l}")
    cx.zdsb = getattr(cx, "zdsb", {})
    cx.zdsb[l] = cx.zp.tile([128, cx.nsub * 4], cx.bf16, tag="zdsb",
                            name=f"zdsb{l}")
    # shared collective outputs, one per (layer, src half): Shared DRAM
    # tiles are single-writer, so each half AG gets its own
    hpr = PR // cx.nch
    for q in range(cx.nch):
        cx.hfull[(l, q)] = cx.dp.tile(
            [NCORES * hpr, HAUG], cx.bf16, name=f"h_aug_full{l}_{q}",
            tag=f"hfull{l}_{q}", addr_space="Shared")


def _emit_dense_tile(nc, cx, l, t):
    """Dense phase for one 128-row tile of layer l; fires the AG chunk when
    this tile completes a chunk boundary."""
    Act, f32, bf16 = cx.Act, cx.f32, cx.bf16
    Alu = cx.Alu
    kch = 2 if l == 0 else 8
    wsb = cx.wsb[l]
    r0 = t * 128
    rowb = cx.iop.tile([128, HAUG], bf16, tag="rowb", name=f"rb{l}_{t}")
    rowbF = rowb[:, :].bitcast(f32)      # [128, 576]

    def xTb(kc):
        return cx.xT_sb[:, t * HC + kc * 128:t * HC + (kc + 1) * 128]

    # h in two 512-col bank generations, then a_s|a_d as a third
    for sl in range(2):
        ph = cx.phd.tile([128, 512], f32, tag="phd", name=f"ph{l}_{t}_{sl}")
        for kc in range(kch):
            nc.tensor.matmul(
                ph[:, :], lhsT=xTb(kc),
                rhs=wsb[:, kc * WA + sl * 512: kc * WA + (sl + 1) * 512],
                start=(kc == 0), stop=(kc == kch - 1))
        nc.scalar.activation(rowb[:, sl * 512:(sl + 1) * 512], ph[:, :], Act.Copy)
    pa = cx.phd.tile([128, 512], f32, tag="phd", name=f"pa{l}_{t}")
    for kc in range(kch):
        nc.tensor.matmul(
            pa[:, 0:8], lhsT=xTb(kc), rhs=wsb[:, kc * WA + HC: kc * WA + WA],
            start=(kc == 0), stop=(kc == kch - 1))
    # raw fp32 a_s into the row (bit-exact via bitcast view)
    nc.vector.tensor_copy(rowbF[:, 512:516], pa[:, 0:4])
    # a_d -> resident bf16 table
    nc.scalar.activation(cx.adloc[l][:, t * 4:(t + 1) * 4], pa[:, 4:8], Act.Copy)
    nc.vector.memset(rowb[:, HC + 8:HAUG], 0.0)
    rbd = nc.sync.dma_start(cx.h_local[r0:r0 + 128, :], rowb[:, :])
    ck = t // cx.tpc
    cx.chunk_dmas.append(rbd)
    if cx.prev_ags[ck] is not None:
        cx.add_dep(rbd.ins, cx.prev_ags[ck].ins,
                   reason="h_local WAR vs prev AG chunk")
    if t % cx.tpc == cx.tpc - 1:
        rows = cx.tpc * 128
        ag = nc.gpsimd.collective_compute(
            "AllGather", cx.Alu.bypass,
            replica_groups=[list(range(NCORES))],
            ins=[cx.h_local[ck * rows:(ck + 1) * rows, :].opt()],
            outs=[cx.hfull[(l, ck)][:, :].opt()],
        )
        for rbd2 in cx.chunk_dmas:
            cx.add_dep(ag.ins, rbd2.ins, reason="AG after h_local chunk writes")
        cx.chunk_dmas = []
        cx.cur_ags[ck] = ag
        cx.prev_ags[ck] = ag


def _emit_prepass_seg(nc, cx, l, si, seg, zdsb):
    """Per-edge a_d via host-shipped transposed one-hot + small matmuls."""
    Act, Alu, f32, bf16 = cx.Act, cx.Alu, cx.f32, cx.bf16
    q, tk, b0 = seg
    nsb = sum(kk for _, kk in tk)
    itg = cx.zp.tile([128, cx.kmax * 128], bf16, tag="itg", name=f"itg{l}_{si}")
    nc.sync.dma_start(itg[:, 0:nsb * 128],
                      cx.ins["indT"][:, b0 * 128:(b0 + nsb) * 128])
    # double-buffered zd slice (group parity) so group si+1's matmuls don't
    # serialize behind group si's zdsb copy
    zb = 128 + (si % 2) * 64
    zd = cx.ps_bank[:, zb:zb + nsb * 4]
    for s4 in range(nsb):
        td = int(cx.tos[b0 + s4])
        nc.tensor.matmul(zd[:, s4 * 4:(s4 + 1) * 4],
                         lhsT=itg[:, s4 * 128:(s4 + 1) * 128],
                         rhs=cx.adloc[l][:, td * 4:(td + 1) * 4],
                         start=True, stop=True)
    nc.scalar.activation(zdsb[:, b0 * 4:(b0 + nsb) * 4], zd, Act.Copy)


def _emit_edge_seg(nc, cx, l, si, seg, zdsb):
    Act, Alu, f32, bf16 = cx.Act, cx.Alu, cx.f32, cx.bf16
    q, tk, sb0 = seg
    nsb = sum(kk for _, kk in tk)
    ifg = cx.gp.tile([128, cx.kmax * 128], bf16, tag="ifg", name=f"if{l}_{si}")
    nc.vector.tensor_tensor(
        ifg[:, 0:nsb * 128].rearrange("p (s j) -> p s j", j=128),
        cx.iota[:, :].unsqueeze(1).broadcast_to((128, nsb, 128)),
        cx.drel[:, sb0:sb0 + nsb]
            .unsqueeze(2).broadcast_to((128, nsb, 128)),
        Alu.is_equal)
    ic = cx.isrc[:, sb0 * 8:(sb0 + nsb) * 8]
    hr = cx.mp.tile([128, cx.kmax * HAUG], bf16, tag="hr", name=f"hr{l}_{si}")
    g3 = nc.gpsimd.dma_gather(
        hr[:, 0:nsb * HAUG].rearrange("p (a b) -> p a b", b=HAUG),
        cx.hfull[(l, q)][:, 0:HAUG], ic, nsb * 128, nsb * 128, HAUG,
        elem_step=HAUG, single_packet=False, queue_num=si % 4)
    cx.add_dep(g3.ins, cx.ags[q].ins, reason="gather after its half AG")

    hrF = hr[:, :].bitcast(f32)    # [128, kmax*576]
    z = cx.sp.tile([128, cx.kmax * 4], f32, tag="z", name=f"z{l}_{si}")
    nc.vector.tensor_tensor(
        z[:, 0:nsb * 4].rearrange("p (a b) -> p a b", b=4),
        hrF.rearrange("p (s c) -> p s c", c=576)[:, 0:nsb, 512:516],
        zdsb[:, sb0 * 4:(sb0 + nsb) * 4].rearrange("p (a b) -> p a b", b=4),
        Alu.add)
    # walpha bf16 [128, nsb*8]: per s4 [w(4) | alpha(4)]
    wal = cx.sp.tile([128, cx.kmax * 8], bf16, tag="wal", name=f"wa{l}_{si}")
    wal3 = wal[:, 0:nsb * 8].rearrange("p (s x) -> p s x", x=8)
    nc.vector.scalar_tensor_tensor(
        wal3[:, :, 4:8],
        z[:, 0:nsb * 4].rearrange("p (s h) -> p s h", h=H),
        NEG, z[:, 0:nsb * 4].rearrange("p (s h) -> p s h", h=H),
        Alu.mult, Alu.max)
    nc.scalar.activation(wal3[:, :, 0:4], wal3[:, :, 4:8], Act.Exp)
    # f32 w for the scalar-engine per-partition scale (second exp, same
    # bf16 alpha input -> bf16(w_f32) == the bf16 w above)
    wf = cx.sp.tile([128, cx.kmax * 4], f32, tag="wf", name=f"wf{l}_{si}")
    nc.scalar.activation(
        wf[:, 0:nsb * 4].rearrange("p (s h) -> p s h", h=H),
        wal3[:, :, 4:8], Act.Exp)

    # per-segment weighted indicators: head 0 on ScalarE (per-partition
    # scale), heads 1-3 in one batched DVE op
    wind = cx.wp2.tile([128, cx.kmax * H * 128], bf16, tag="wind",
                       name=f"wi{l}_{si}")
    wind4 = wind[:, 0:nsb * H * 128].rearrange("p (s h j) -> p s h j",
                                               h=H, j=128)
    nc.vector.tensor_tensor(
        wind4[:, :, 1:4, :],
        ifg[:, 0:nsb * 128].rearrange("p (s j) -> p s j", j=128)
            .unsqueeze(2).broadcast_to((128, nsb, 3, 128)),
        wal3[:, :, 1:4].unsqueeze(3).broadcast_to((128, nsb, 3, 128)),
        Alu.mult)
    for s4 in range(nsb):
        nc.scalar.activation(
            wind[:, s4 * H * 128:s4 * H * 128 + 128],
            ifg[:, s4 * 128:(s4 + 1) * 128], Act.Copy,
            scale=wf[:, s4 * 4:s4 * 4 + 1])

    # per-tile runs within the segment (tk gives tile order + counts)
    s4a = 0
    for ri, (t, kk) in enumerate(tk):
        s4b = s4a + kk - 1
        if t not in cx.agg:
            cx.agg[t] = cx.pbig.tile([128, HC], f32, tag="pbig",
                                     name=f"agg{l}_{t}")
        P = cx.agg[t]
        Pa = cx.walsum[:, (t % 2) * 8:(t % 2) * 8 + 8]
        fi = cx.first_of[t] == sb0 + s4a
        la = cx.last_of[t] == sb0 + s4b
        # [w|alpha] partial for the whole run (own PSUM accumulation in the
        # scratch bank), then one DVE accumulate into the walsum parity slot
        pp = cx.ps_bank[:, 64 + (ri % 2) * 8:72 + (ri % 2) * 8]
        for s4 in range(s4a, s4b + 1):
            b0 = s4 * HAUG
            # 2 heads share a 2KB PSUM zero-region (bank): only the first
            # matmul per bank may carry start, only the last may carry stop.
            for hd in range(H):
                nc.tensor.matmul(
                    P[:, hd * C:(hd + 1) * C],
                    lhsT=wind[:, s4 * H * 128 + hd * 128:
                              s4 * H * 128 + (hd + 1) * 128],
                    rhs=hr[:, b0 + hd * C:b0 + (hd + 1) * C],
                    start=fi and s4 == s4a and hd % 2 == 0,
                    stop=la and s4 == s4b and hd % 2 == 1)
            nc.tensor.matmul(pp, lhsT=ifg[:, s4 * 128:(s4 + 1) * 128],
                             rhs=wal[:, s4 * 8:(s4 + 1) * 8],
                             start=s4 == s4a, stop=s4 == s4b)
        if fi:
            nc.vector.tensor_copy(Pa, pp)
        else:
            nc.vector.tensor_tensor(Pa, Pa, pp, Alu.add)
        if la:
            _finalize(nc, cx, l, t, P, Pa)
            del cx.agg[t]
            if l + 1 < cx.nlayers:
                _emit_dense_tile(nc, cx, l + 1, t)
        s4a = s4b + 1


def _finalize(nc, cx, l, t, P, Pa):
    Alu, Act = cx.Alu, cx.Act
    f32, bf16 = cx.f32, cx.bf16
    sp, fp = cx.sp, cx.fp
    r0 = t * 128
    t1 = sp.tile([128, 4], f32, tag="t1", name=f"t1{l}_{t}")
    nc.scalar.activation(t1[:, :], Pa[:, 4:8], Act.Exp, scale=-1.0)
    ts = sp.tile([128, 4], f32, tag="ts", name=f"ts{l}_{t}")
    nc.vector.tensor_tensor(ts[:, :], t1[:, :], Pa[:, 0:4], Alu.mult)
    nc.vector.tensor_scalar_add(ts[:, :], ts[:, :], 1e-16)
    rc = sp.tile([128, 4], f32, tag="rc", name=f"rc{l}_{t}")
    nc.vector.reciprocal(rc[:, :], ts[:, :])
    cf = sp.tile([128, 4], f32, tag="cf", name=f"cf{l}_{t}")
    nc.vector.tensor_tensor(cf[:, :], t1[:, :], rc[:, :], Alu.mult)
    outb = fp.tile([128, HC], f32, tag="outb", name=f"ob{l}_{t}")
    nc.vector.tensor_tensor(
        outb[:, :].rearrange("p (h c) -> p h c", c=C),
        P[:, 0:HC].rearrange("p (h c) -> p h c", c=C),
        cf[:, :].unsqueeze(2).broadcast_to((128, H, C)), Alu.mult)
    if cx.has_bias:
        nc.vector.tensor_tensor(outb[:, :], outb[:, :], cx.brep[l][:, :], Alu.add)
    relu = fp.tile([128, HC], bf16, tag="relu", name=f"rl{l}_{t}")
    nc.scalar.activation(relu[:, :], outb[:, :], Act.Relu)
    def pe_transpose_to(dst_of_kc, nm):
        # PE-transpose via the phd scratch banks; psum->sbuf copies split
        # across ScalarE and DVE
        for kc in range(8):
            pt = cx.phd.tile([128, 512], cx.f32, tag="phd", name=f"{nm}_{kc}")
            ptw = pt[:, 0:64].bitcast(bf16)
            nc.tensor.transpose(ptw, relu[:, kc * 128:(kc + 1) * 128],
                                cx.ident[:, :])
            if kc % 2 == 0:
                nc.scalar.activation(dst_of_kc(kc), ptw, Act.Copy)
            else:
                nc.vector.tensor_copy(dst_of_kc(kc), ptw)

    if l < cx.nlayers - 1:
        # PE-transpose into the resident next-layer lhsT (no DRAM round trip)
        pe_transpose_to(
            lambda kc: cx.xT_sb[:, t * HC + kc * 128:t * HC + (kc + 1) * 128],
            f"xt{l}_{t}")
        return
    # final layer: fc head + row softmax
    hT = fp.tile([128, HC], bf16, tag="hT", name=f"hT{t}")
    pe_transpose_to(lambda kc: hT[:, kc * 128:(kc + 1) * 128], f"ft{t}")
    plt = cx.phd.tile([128, 512], f32, tag="phd", name=f"pl{t}")
    pl = plt[:, 0:16]
    for kc in range(8):
        nc.tensor.matmul(pl[:, 0:OUT], lhsT=hT[:, kc * 128:(kc + 1) * 128],
                         rhs=cx.fcw[:, kc * OUT:(kc + 1) * OUT],
                         start=(kc == 0), stop=(kc == 7))
    lg = sp.tile([128, OUT], f32, tag="lg", name=f"lg{t}")
    nc.vector.tensor_tensor(lg[:, :], pl[:, 0:OUT], cx.fcb[:, :], Alu.add)
    nc.sync.dma_start(cx.logits_o[r0:r0 + 128, :], lg[:, :])
    mx = sp.tile([128, 1], f32, tag="mx", name=f"mx{t}")
    nc.vector.tensor_reduce(mx[:, :], lg[:, :], cx.mybir.AxisListType.X, Alu.max)
    l2 = sp.tile([128, OUT], f32, tag="l2", name=f"l2{t}")
    nc.vector.tensor_scalar_sub(l2[:, :], lg[:, :], mx[:, 0:1])
    ex = sp.tile([128, OUT], f32, tag="ex", name=f"ex{t}")
    se = sp.tile([128, 1], f32, tag="se", name=f"se{t}")
    nc.scalar.activation(ex[:, :], l2[:, :], Act.Exp, accum_out=se[:, :])
    rs = sp.tile([128, 1], f32, tag="rs", name=f"rs{t}")
    nc.vector.reciprocal(rs[:, :], se[:, :])
    pb = sp.tile([128, OUT], f32, tag="pb", name=f"pb{t}")
    nc.vector.tensor_scalar_mul(pb[:, :], ex[:, :], rs[:, 0:1])
    nc.sync.dma_start(cx.probs_o[r0:r0 + 128, :], pb[:, :])


_CACHE = {}


def _build_and_run(inputs, trace=False):
    from concourse import bacc, tile, mybir
    from concourse.bass_utils import run_bass_kernel_spmd

    in_maps, inv, tile_of_sub, segs, nsub, has_bias = _prep_inputs(inputs)
    key = (repr(segs), nsub, has_bias, tuple(int(t) for t in tile_of_sub))
    if key not in _CACHE:
        nc = bacc.Bacc("TRN2", target_bir_lowering=False, debug=False,
                       enable_asserts=False, num_devices=NCORES,
                       num_swdge_queues=4)
        build_program(nc, tile, mybir, tile_of_sub, segs, nsub,
                      has_bias=has_bias)
        _CACHE[key] = nc
    nc = _CACHE[key]
    res = run_bass_kernel_spmd(nc, in_maps, list(range(NCORES)), trace=trace)
    probs = np.empty((N, OUT), np.float32)
    logits = np.empty((N, OUT), np.float32)
    for c in range(NCORES):
        probs[c * RPC:(c + 1) * RPC] = res.results[c]["probs"][inv[c]]
        logits[c * RPC:(c + 1) * RPC] = res.results[c]["logits"][inv[c]]
    return (probs, logits), res


def kernel(**inputs):
    (probs, logits), _ = _build_and_run(inputs, trace=False)
    return probs, logits
